# revision 34
# baseline (speedup 1.0000x reference)
"""Trainium2 Bass kernel for causal multi-head attention.

Problem: B=4, S=2048, D=1024, H=16 (head_dim 64), fp32.
  qkv = x @ w_attn + b_attn ; causal SDPA ; out @ w_proj + b_proj

Sharding (8 cores): data-parallel over B (4) x tensor-parallel over head
halves (2). Core c handles batch b=c//2, heads [8*(c%2), 8*(c%2)+8).
Each core computes its qkv slice, its heads' attention, and a partial
output projection (its heads' rows of w_proj); the host sums the two
partials per batch. b_proj is added on even cores (odd cores get zeros).

All matmul inputs are bf16 (rounded on host / at PSUM evacuation);
accumulation stays fp32 in PSUM, so the only error sources are input
rounding (~0.1% rms/element), well inside the 2e-2 rel-err budget.

Device dataflow (per core), emitted as one fused pipeline:
  P1 (chunked over s in 4 chunks of 512):
      qT,kT = (w_q|k)^T @ x^T  -> [e, s] (e on partitions), bias+scale
      fused into the PSUM evacuation; v natural [s, e], stored augmented
      as [ones | v_h] per head so a single M=128 attn@v matmul also
      yields the softmax denominator.  Weights live in SBUF in an
      et-major layout so the first matmul only needs a 256KB DMA.
  P2 (query blocks g of 512, interleaved with P1 chunks g+1.. and P3):
      per head-pair p, key tile t (128): scoresT = kT.T @ qT, exp on ACT
      (no max subtraction: scores are O(+-6)), block-causal via narrowed
      moving ranges + triangular multiply on diagonal tiles, then
      [den | outT] += [ones | v].T @ expT.  The exp on the ACT engine is
      the P2 pacer (~0.85ns/elem), so P1/P3 matmul groups are
      interleaved between attention tiles as PE filler.
  P3: y_partial = aoT.T @ w_proj_slice (+ b_proj), queued per query
      block and drained as filler during the next block.
"""

import math
import os
from collections import deque

import ml_dtypes
import numpy as np

import concourse.bass as bass
import concourse.mybir as mybir
import concourse.tile as tile
from concourse import bacc

last_exec_time_ns = None

B, S, D, H = 4, 2048, 1024, 16
HD = D // H          # 64
HPC = H // 2         # heads per core = 8
EC = HPC * HD        # per-core qkv slice width = 512
NP = 4               # head pairs per core
QB = 512             # query block width
KT = 128             # key tile
CB = 512             # P1 s-chunk width
NC_ = S // CB        # 4 chunks
N_KT = S // KT       # 16
DT = D // 128        # 8 contraction tiles

F32 = mybir.dt.float32
BF16 = mybir.dt.bfloat16
NPBF = ml_dtypes.bfloat16

_nc_cache: dict = {}


def _build(causal: bool):
    nc = bacc.Bacc("TRN2", target_bir_lowering=False)
    xp = nc.dram_tensor("xp", [NC_, 128, DT, CB], BF16, kind="ExternalInput")
    wqk = nc.dram_tensor("wqk", [8, 128, DT, 128], BF16, kind="ExternalInput")
    wv = nc.dram_tensor("wv", [DT, 128, EC], BF16, kind="ExternalInput")
    wpj = nc.dram_tensor("wpj", [4, 128, D], BF16, kind="ExternalInput")
    bqk = nc.dram_tensor("bqk", [128, 8], F32, kind="ExternalInput")
    bv = nc.dram_tensor("bv", [1, EC], F32, kind="ExternalInput")
    bp = nc.dram_tensor("bp", [128, 8], F32, kind="ExternalInput")
    tri = nc.dram_tensor("tri", [128, 128], BF16, kind="ExternalInput")
    # y is produced transposed ([d, s]) in bf16; the host transposes back.
    # This puts d on partitions during the output projection, so the bias
    # add rides the ACT-engine evacuation (bias is per-partition there).
    y = nc.dram_tensor("y", [D, S], BF16, kind="ExternalOutput")

    scale = 1.0 / math.sqrt(HD)
    LOOK = 2  # score/exp tiles emitted ahead of attn@v

    with tile.TileContext(nc) as tc, nc.allow_low_precision(
        reason="bf16 attention pipeline; fp32 PSUM accumulation throughout"
    ):
        with (
            tc.tile_pool(name="sb", bufs=1) as sb,
            tc.tile_pool(name="eABp", bufs=4) as eABp,
            tc.tile_pool(name="ysbp", bufs=3) as ysbp,
            tc.tile_pool(name="rcpp", bufs=2) as rcpp,
            tc.tile_pool(name="p1ps", bufs=2, space="PSUM") as p1ps,
            tc.tile_pool(name="psS", bufs=2, space="PSUM") as psS,
            tc.tile_pool(name="psO", bufs=1, space="PSUM") as psO,
        ):
            w_qk = sb.tile([128, 8, DT, 128], BF16, tag="w_qk")
            w_v = sb.tile([128, DT, EC], BF16, tag="w_v")
            # [e-in-eo, eo, dcol, d-in-dcol]: stationary tiles for the
            # transposed output projection
            wp_sb = sb.tile([128, 4, DT, 128], BF16, tag="wp_sb")
            xts = sb.tile([128, NC_, DT, CB], BF16, tag="xts")
            qT = sb.tile([128, NP, S], BF16, tag="qT")
            kT = sb.tile([128, NP, S], BF16, tag="kT")
            # augmented v: per head h and key tile t, [ones | v_h] so one
            # M=128 matmul yields both attn@v and the softmax denominator
            va = sb.tile([128, N_KT, HPC, 128], BF16, tag="va")
            aoT = sb.tile([128, NP, S], BF16, tag="aoT")
            bqk_sb = sb.tile([128, 8], F32, tag="bqk_sb")
            bv_sb = sb.tile([128, EC], F32, tag="bv_sb")
            bp_sb = sb.tile([128, 8], F32, tag="bp_sb")
            tri_sb = sb.tile([128, 128], BF16, tag="tri_sb")

            # ---- prologue DMAs: the startup-critical stream rides the sync
            # queue in need-order (gpsimd's DMA path has ~8us engine startup
            # and ~0.85us per issue; the scalar queue must stay empty so the
            # first exp isn't stuck behind input transfers) ----
            # Startup-critical stream: few LARGE DMAs (transfers stripe over
            # 16 DMA engines, so big pieces get full bandwidth; small pieces
            # serialize on the ~4-outstanding queue limit). Scalar carries
            # only pieces that retire before its first exp (~23us).
            nc.scalar.dma_start(out=w_qk[:, 0], in_=wqk.ap()[0])
            nc.sync.dma_start(out=xts[:, 0, 0:4], in_=xp.ap()[0][:, 0:4])
            nc.sync.dma_start(out=xts[:, 0, 4:8], in_=xp.ap()[0][:, 4:8])
            for et in range(1, 5):
                nc.sync.dma_start(out=w_qk[:, et], in_=wqk.ap()[et])
            for et in range(5, 8):
                nc.scalar.dma_start(out=w_qk[:, et], in_=wqk.ap()[et])
            nc.scalar.dma_start(out=bqk_sb, in_=bqk.ap())
            nc.scalar.dma_start(out=tri_sb, in_=tri.ap())
            nc.scalar.dma_start(out=bv_sb, in_=bv.ap().to_broadcast([128, EC]))
            for dt in range(DT):
                eng = nc.sync if dt < 4 else nc.scalar
                eng.dma_start(out=w_v[:, dt], in_=wv.ap()[dt])
            # gpsimd: nothing here is needed before ~20us
            nc.gpsimd.memset(va[:, 0:4, :, 0:64], 1.0)
            for c in range(1, NC_):
                nc.gpsimd.dma_start(out=xts[:, c], in_=xp.ap()[c])
                nc.gpsimd.memset(va[:, 4 * c:4 * c + 4, :, 0:64], 1.0)
            for eo in range(4):
                nc.gpsimd.dma_start(out=wp_sb[:, eo], in_=wpj.ap()[eo])
            nc.gpsimd.dma_start(out=bp_sb, in_=bp.ap())

            # ---------------- P1: one chunk = 12 matmul groups ----------------
            # chunk 0's q/k groups run in DMA-arrival order (et0,5-7 land
            # first via the scalar queue)
            ET_ORDER0 = (0, 5, 6, 7, 1, 2, 3, 4)

            def p1_groups(c, et_order=tuple(range(8))):
                gs = []
                for et in et_order:
                    def qk_group(et=et, c=c):
                        ps = p1ps.tile([128, CB], F32, tag="P1", name=f"qk{c}_{et}")
                        for dt in range(DT):
                            nc.tensor.matmul(
                                ps,
                                w_qk[:, et, dt, :],
                                xts[:, c, dt, :],
                                start=(dt == 0),
                                stop=(dt == DT - 1),
                            )
                        dst, slab = (qT, et) if et < 4 else (kT, et - 4)
                        nc.vector.tensor_scalar(
                            out=dst[:, slab, c * CB:(c + 1) * CB],
                            in0=ps,
                            scalar1=bqk_sb[:, et:et + 1],
                            scalar2=scale if et < 4 else 1.0,
                            op0=mybir.AluOpType.add,
                            op1=mybir.AluOpType.mult,
                        )
                    gs.append(qk_group)
                for st in range(CB // 128):
                    def v_group(st=st, c=c):
                        ps = p1ps.tile([128, EC], F32, tag="P1", name=f"v{c}_{st}")
                        for dt in range(DT):
                            nc.tensor.matmul(
                                ps,
                                xts[:, c, dt, st * 128:(st + 1) * 128],
                                w_v[:, dt, :],
                                start=(dt == 0),
                                stop=(dt == DT - 1),
                            )
                        nc.vector.tensor_tensor(
                            out=va[:, c * 4 + st, :, 64:128],
                            in0=ps.rearrange("p (h e) -> p h e", e=64),
                            in1=bv_sb.rearrange("p (h e) -> p h e", e=64),
                            op=mybir.AluOpType.add,
                        )
                    gs.append(v_group)
                return gs

            # ---------------- P3: one group = 4 matmuls + ACT evac/DMA --------
            # transposed: py[dcol partitions, q free]; bias add + bf16 cast
            # ride the ACT-engine evacuation (bias is per-partition here)
            def emit_p3(qs, ql, dcol, on_act=False):
                py = p1ps.tile([128, QB], F32, tag="P1", name=f"p3_{qs}_{dcol}")
                for eo in range(4):
                    nc.tensor.matmul(
                        py[:, 0:ql],
                        wp_sb[:, eo, dcol, :],
                        aoT[:, eo, qs:qs + ql],
                        start=(eo == 0),
                        stop=(eo == 3),
                    )
                ysb = ysbp.tile([128, QB], BF16, tag="ysb", name=f"ysb{qs}_{dcol}")
                if on_act:
                    # tail only: ACT is idle once the last exps are done
                    nc.scalar.activation(
                        ysb[:, 0:ql], py[:, 0:ql],
                        mybir.ActivationFunctionType.Identity,
                        bias=bp_sb[:, dcol:dcol + 1],
                    )
                else:
                    nc.vector.tensor_scalar(
                        out=ysb[:, 0:ql], in0=py[:, 0:ql],
                        scalar1=bp_sb[:, dcol:dcol + 1],
                        scalar2=1.0,
                        op0=mybir.AluOpType.add,
                        op1=mybir.AluOpType.mult,
                    )
                nc.sync.dma_start(
                    out=y.ap()[dcol * 128:(dcol + 1) * 128, qs:qs + ql],
                    in_=ysb[:, 0:ql],
                )

            fillers = deque()   # pending P1 groups (callables)
            p3q = deque()       # pending P3 (st, dh) groups

            def pop_filler():
                if fillers:
                    fillers.popleft()()
                    return True
                if p3q:
                    emit_p3(*p3q.popleft())
                    return True
                return False

            def queue_p3(qs, ql):
                for dcol in range(DT):
                    p3q.append((qs, ql, dcol))

            tri_b = None

            def get_tri_b():
                nonlocal tri_b
                if tri_b is None:
                    tri_b = bass.AP(
                        tensor=tri_sb.tensor,
                        offset=tri_sb.offset,
                        ap=[tri_sb.ap[0], [0, 2], tri_sb.ap[1]],
                    )
                return tri_b

            # ---------------- P2: attention pair with PE filler ----------------
            def emit_pair(g, p, quota, qa=0, qb=QB):
                # queries [g*QB + qa, g*QB + qb); a (qa, qb) sub-range is
                # used to split the final pair so its P3 can start earlier
                q0 = g * QB
                if causal:
                    tiles = [t for t in range(4 * (g + 1))
                             if 128 * (t - 4 * g) < qb]
                else:
                    tiles = list(range(N_KT))
                n_t = len(tiles)
                OA = psO.tile([128, QB], F32, tag="OA")
                OB = psO.tile([128, QB], F32, tag="OB")

                def emit_score_exp(t):
                    j = t - 4 * g if causal else -1
                    qlo = max(qa, 128 * j) if j >= 0 else qa
                    tri_at = 128 * j if (j >= 0 and 128 * j >= qa) else -1
                    SAB = psS.tile([128, 2, QB], F32, tag="SAB")
                    k0 = t * KT
                    nc.tensor.matmul(
                        SAB[:, 0, qlo:qb],
                        kT[0:64, p, k0:k0 + KT],
                        qT[0:64, p, q0 + qlo:q0 + qb],
                        start=True, stop=True,
                    )
                    nc.tensor.matmul(
                        SAB[:, 1, qlo:qb],
                        kT[64:128, p, k0:k0 + KT],
                        qT[64:128, p, q0 + qlo:q0 + qb],
                        start=True, stop=True,
                    )
                    eAB = eABp.tile([128, 2, QB], BF16, tag="eAB")
                    nc.scalar.activation(
                        eAB[:, :, qlo:qb], SAB[:, :, qlo:qb],
                        mybir.ActivationFunctionType.Exp,
                    )
                    if tri_at >= 0:
                        nc.vector.tensor_tensor(
                            out=eAB[:, :, tri_at:tri_at + 128],
                            in0=eAB[:, :, tri_at:tri_at + 128],
                            in1=get_tri_b(),
                            op=mybir.AluOpType.mult,
                        )
                    return qlo, eAB

                def emit_av(ti, qlo, eAB):
                    t = tiles[ti]
                    nc.tensor.matmul(
                        OA[:, qlo:qb],
                        va[:, t, 2 * p, :],
                        eAB[:, 0, qlo:qb],
                        start=(ti == 0), stop=(ti == n_t - 1),
                    )
                    nc.tensor.matmul(
                        OB[:, qlo:qb],
                        va[:, t, 2 * p + 1, :],
                        eAB[:, 1, qlo:qb],
                        start=(ti == 0), stop=(ti == n_t - 1),
                    )

                done = 0
                stride = max(1, n_t // quota) if quota else n_t + 1
                pending = []
                for ti, t in enumerate(tiles):
                    pending.append((ti, *emit_score_exp(t)))
                    if done < quota and ti % stride == stride - 1:
                        if pop_filler():
                            done += 1
                    if len(pending) > LOOK:
                        emit_av(*pending.pop(0))
                for item in pending:
                    emit_av(*item)
                while done < quota and pop_filler():
                    done += 1

                rcpA = rcpp.tile([64, QB], F32, tag="rcpA")
                rcpB = rcpp.tile([64, QB], F32, tag="rcpB")
                nc.vector.reciprocal_approx_fast(
                    out=rcpA[:, qa:qb], in_=OA[0:64, qa:qb]
                )
                nc.vector.reciprocal_approx_fast(
                    out=rcpB[:, qa:qb], in_=OB[0:64, qa:qb]
                )
                nc.vector.tensor_tensor(
                    out=aoT[0:64, p, q0 + qa:q0 + qb],
                    in0=OA[64:128, qa:qb],
                    in1=rcpA[:, qa:qb],
                    op=mybir.AluOpType.mult,
                )
                nc.vector.tensor_tensor(
                    out=aoT[64:128, p, q0 + qa:q0 + qb],
                    in0=OB[64:128, qa:qb],
                    in1=rcpB[:, qa:qb],
                    op=mybir.AluOpType.mult,
                )

            # ---------------- main schedule ----------------
            for fn in p1_groups(0, et_order=ET_ORDER0):
                fn()
            if not causal:
                # every block needs all keys: run P1 fully first
                for c in range(1, NC_):
                    for fn in p1_groups(c):
                        fn()
            last = NC_ - 1
            # per-block quota caps roll P3 filler into the ACT-tight last
            # block, where the PE would otherwise wait on exp
            caps = [3, 4, 3, 99]
            for g in range(NC_):
                if causal and g < last:
                    # just-in-time: chunk g+1 is interleaved into block g
                    fillers.extend(p1_groups(g + 1))
                for p in range(NP):
                    remaining = len(fillers) + len(p3q)
                    quota = min(-(-remaining // (NP - p)), caps[g])  # ceil
                    if g == last and p == NP - 1:
                        # split the final pair so the last block's output
                        # projection overlaps its second half
                        emit_pair(g, p, quota, 0, QB // 2)
                        queue_p3(g * QB, QB // 2)
                        emit_pair(g, p, len(p3q), QB // 2, QB)
                        queue_p3(g * QB + QB // 2, QB // 2)
                    else:
                        emit_pair(g, p, quota)
                if g != last:
                    queue_p3(g * QB, QB)
            ti = 0
            while p3q:
                # final drain: split evacuations between ACT (idle now) and DVE
                emit_p3(*p3q.popleft(), on_act=(ti % 2 == 0))
                ti += 1

    nc.compile()
    return nc


def _get_nc(causal: bool):
    if causal not in _nc_cache:
        _nc_cache[causal] = _build(causal)
    return _nc_cache[causal]


def _numpy_fallback(x, mask, w_attn, b_attn, w_proj, b_proj):
    x64 = x.astype(np.float64)
    qkv = x64 @ w_attn.astype(np.float64) + b_attn.astype(np.float64)
    q, k, v = np.split(qkv, 3, axis=-1)
    sp = lambda t: t.reshape(B, S, H, HD).transpose(0, 2, 1, 3)
    q, k, v = sp(q), sp(k), sp(v)
    scores = np.einsum("bhqd,bhkd->bhqk", q, k) / math.sqrt(HD)
    m = np.broadcast_to(np.asarray(mask, bool), scores.shape)
    scores = np.where(m, scores, -np.inf)
    scores -= scores.max(axis=-1, keepdims=True)
    e = np.exp(scores)
    attn = e / e.sum(axis=-1, keepdims=True)
    out = np.einsum("bhqk,bhkd->bhqd", attn, v)
    out = out.transpose(0, 2, 1, 3).reshape(B, S, D)
    return (out @ w_proj.astype(np.float64) + b_proj.astype(np.float64)).astype(
        np.float32
    )


def kernel(x, mask, w_attn, b_attn, w_proj, b_proj) -> np.ndarray:
    from concourse.bass_utils import run_bass_kernel_spmd

    x = np.asarray(x, dtype=np.float32)
    w_attn = np.asarray(w_attn, dtype=np.float32)
    b_attn = np.asarray(b_attn, dtype=np.float32)
    w_proj = np.asarray(w_proj, dtype=np.float32)
    b_proj = np.asarray(b_proj, dtype=np.float32)

    m2 = np.asarray(mask, dtype=bool).reshape(S, S)
    if np.array_equal(m2, np.tril(np.ones((S, S), dtype=bool))):
        causal = True
    elif m2.all():
        causal = False
    else:
        return _numpy_fallback(x, mask, w_attn, b_attn, w_proj, b_proj)

    nc = _get_nc(causal)

    tri_np = np.triu(np.ones((128, 128), dtype=np.float32)).astype(NPBF)

    in_maps = []
    for c in range(8):
        b, hg = divmod(c, 2)
        e0 = hg * EC
        q_sl = slice(e0, e0 + EC)
        k_sl = slice(D + e0, D + e0 + EC)
        v_sl = slice(2 * D + e0, 2 * D + e0 + EC)
        wqk_cat = np.concatenate([w_attn[:, q_sl], w_attn[:, k_sl]], axis=1)
        # [et][p][dt][e]: stationary tile (et, dt) rows = d in dt, cols = e
        wqk_p = np.ascontiguousarray(
            wqk_cat.reshape(DT, 128, 8, 128).transpose(2, 1, 0, 3)
        ).astype(NPBF)
        wv_p = np.ascontiguousarray(
            w_attn[:, v_sl].reshape(DT, 128, EC)
        ).astype(NPBF)
        wp_p = np.ascontiguousarray(
            w_proj[q_sl, :].reshape(4, 128, D)
        ).astype(NPBF)
        # [chunk][p][dt][s]
        xp_p = np.ascontiguousarray(
            x[b].T.reshape(DT, 128, NC_, CB).transpose(2, 1, 0, 3)
        ).astype(NPBF)
        # device evac computes (q_psum + bias) * scale for q tiles
        bqk_np = np.concatenate([b_attn[q_sl], b_attn[k_sl]]).reshape(8, 128).T
        in_maps.append({
            "xp": xp_p,
            "wqk": wqk_p,
            "wv": wv_p,
            "wpj": wp_p,
            "bqk": np.ascontiguousarray(bqk_np, dtype=np.float32),
            "bv": b_attn[v_sl].reshape(1, EC).astype(np.float32),
            "bp": np.ascontiguousarray(
                (b_proj if hg == 0 else np.zeros_like(b_proj))
                .reshape(DT, 128).T.astype(np.float32)
            ),
            "tri": tri_np,
        })

    trace = os.environ.get("KERNEL_TRACE") == "1"
    res = run_bass_kernel_spmd(nc, in_maps, core_ids=list(range(8)), trace=trace)
    global last_exec_time_ns
    if res.exec_time_ns is not None:
        last_exec_time_ns = res.exec_time_ns
    parts = [res.results[c]["y"] for c in range(8)]
    out = np.empty((B, S, D), dtype=np.float32)
    for b in range(B):
        # device output is [d, s] bf16; sum the two head-half partials
        # and transpose back
        out[b] = (
            parts[2 * b].astype(np.float32) + parts[2 * b + 1].astype(np.float32)
        ).T
    return out
